# revision 11
# baseline (speedup 1.0000x reference)
"""Trainium2 Bass kernel for nn_LowRankChristoffel (8 NeuronCores, data parallel).

Reference computation (per row b of v/x/force, D=256, R=16):
    proj  = v @ U                           [B, R]
    scale = 1 / (1 + ||proj|| + eps)
    gamma = (proj^2 * scale) @ W.T          [B, D]
    gate  = x @ Wf.T + bf + force @ Wi.T    [B, D]
    mu    = sigmoid(gate) * 0.1 * (1 + 0.1 * ||v|| / (sqrt(D) + eps))
    out   = 10 * tanh((gamma + mu * v) / 10)

Strategy:
  - Pure data parallel over 8 cores (batch 262144 -> 32768 rows/core).
  - Host pre-transposes v/x/force shards to [D, rows] (and casts to fp16)
    so every SBUF tile arrives in [dim, row] layout (contraction dim on
    partitions) with zero on-chip transposes; output is produced
    transposed (f32) and un-transposed on the host.
  - All matmuls in fp16 (1 cyc/row), accumulating in f32 PSUM.
  - Per-row statistics (||v||^2 rows G..2G-1, ||proj||^2 rows 0..G-1) are
    accumulated into one batched [2G, 512] PSUM tile via "select-column"
    matmuls; a single Sqrt + reciprocal chain runs once per G-block group
    (exactly 2 ACT table loads per group: sqrt set <-> sigmoid/tanh set).
  - Per-row scalars are broadcast to [*, 512] tiles with row-select
    matmuls straight from the batched stats tiles.
"""

import sys
import types

import numpy as np

sys.path.insert(0, "/opt/trn_rl_repo")

import concourse.mybir as mybir
import concourse.tile as tile
from concourse import bacc


def _install_ntff_shim():
    """Register the NTFF profile hook that trn_boot skips when
    antenv.axon_hooks is absent from the image (needed for trace=True)."""
    if "antenv.axon_hooks" in sys.modules:
        return
    try:
        from trn_agent_boot.trn_boot import _ntff_profile_via_ctypes

        hook = _ntff_profile_via_ctypes("/opt/axon/libaxon_pjrt.so")
    except Exception:
        hook = None
    mod = types.ModuleType("antenv.axon_hooks")
    mod._hook = hook
    mod.get_axon_ntff_profile_hook = lambda: mod._hook
    mod.set_axon_ntff_profile_hook = lambda h: setattr(mod, "_hook", h)
    sys.modules["antenv.axon_hooks"] = mod


_install_ntff_shim()
import ml_dtypes
from concourse.bass_utils import run_bass_kernel_spmd

NCORES = 8
D = 256
R = 16
BATCH = 262144
ROWS_PER_CORE = BATCH // NCORES  # 32768
NB = 512  # rows per block (matmul moving-dim)
XF_BATCH = 4  # blocks per x/force DMA
OUT_BATCH = 4  # blocks per output DMA

CLAMP = 10.0
FRICTION_SCALE = 0.1
EPS = 1e-6
VEL_SCALE = 0.1

F32 = mybir.dt.float32
BF16 = mybir.dt.float16  # 16-bit compute dtype (fp16: 11-bit mantissa)


def build(rows_per_core: int, group: int):
    """Build the Bass module for one core (SPMD across all cores)."""
    nblk = rows_per_core // NB
    assert nblk % group == 0
    ngroups = nblk // group
    assert group % XF_BATCH == 0 and group % OUT_BATCH == 0
    G2 = 2 * group
    assert G2 <= 128

    nc = bacc.Bacc(None, target_bir_lowering=False)

    vt = nc.declare_dram_parameter("vt", [D, rows_per_core], BF16, isOutput=False)
    xt = nc.declare_dram_parameter("xt", [D, rows_per_core], BF16, isOutput=False)
    ft = nc.declare_dram_parameter("ft", [D, rows_per_core], BF16, isOutput=False)
    wft = nc.declare_dram_parameter("wft", [D, D], BF16, isOutput=False)
    wit = nc.declare_dram_parameter("wit", [D, D], BF16, isOutput=False)
    u_p = nc.declare_dram_parameter("u", [D, R], BF16, isOutput=False)
    wg = nc.declare_dram_parameter("wg", [R, D], BF16, isOutput=False)
    bf = nc.declare_dram_parameter("bf", [D], F32, isOutput=False)
    selp = nc.declare_dram_parameter("selp", [R, G2 * group], BF16, isOutput=False)
    selv = nc.declare_dram_parameter("selv", [128, G2 * group], BF16, isOutput=False)
    bsels = nc.declare_dram_parameter("bsels", [G2, R * group], BF16, isOutput=False)
    bselv = nc.declare_dram_parameter("bselv", [G2, 128 * group], BF16, isOutput=False)
    outt = nc.declare_dram_parameter("outt", [D, rows_per_core], BF16, isOutput=True)

    AF = mybir.ActivationFunctionType
    OP = mybir.AluOpType
    VEL_A = float(FRICTION_SCALE * VEL_SCALE / (np.sqrt(D) + EPS))
    VEL_B = float(FRICTION_SCALE)

    with tile.TileContext(nc) as tc:
        with (
            tc.tile_pool(name="wpool", bufs=1) as wpool,
            tc.tile_pool(name="vpool", bufs=3) as vpool,
            tc.tile_pool(name="xfpool", bufs=2) as xfpool,
            tc.tile_pool(name="sqp", bufs=group + 2) as sqp_pool,
            tc.tile_pool(name="sqsp", bufs=3) as sqs_pool,
            tc.tile_pool(name="v2p", bufs=4) as v2_pool,
            tc.tile_pool(name="sigp", bufs=3) as sig_pool,
            tc.tile_pool(name="mup", bufs=3) as mu_pool,
            tc.tile_pool(name="outp", bufs=3) as out_pool,
            tc.tile_pool(name="stp", bufs=4) as st_pool,
            tc.tile_pool(name="ps_stats", bufs=1, space="PSUM") as ps_stats,
            tc.tile_pool(name="ps_proj", bufs=2, space="PSUM") as ps_proj,
            tc.tile_pool(name="ps_mm", bufs=3, space="PSUM") as ps_mm,
            tc.tile_pool(name="ps_bc16", bufs=1, space="PSUM") as ps_bc16,
            tc.tile_pool(name="ps_bcv", bufs=1, space="PSUM") as ps_bcv,
        ):
            # ---- load weights / constants once ----
            wft_s = wpool.tile([128, 2, D], BF16)
            nc.sync.dma_start(out=wft_s[:], in_=wft.rearrange("(c p) m -> p c m", p=128))
            wit_s = wpool.tile([128, 2, D], BF16)
            nc.sync.dma_start(out=wit_s[:], in_=wit.rearrange("(c p) m -> p c m", p=128))
            u_s = wpool.tile([128, 2, R], BF16)
            nc.sync.dma_start(out=u_s[:], in_=u_p.rearrange("(c p) r -> p c r", p=128))
            wg_s = wpool.tile([R, D], BF16)
            nc.sync.dma_start(out=wg_s[:], in_=wg[:])
            bf_s = wpool.tile([128, 2], F32)
            nc.sync.dma_start(out=bf_s[:], in_=bf.rearrange("(c p) -> p c", p=128))
            selp_s = wpool.tile([R, G2 * group], BF16)
            nc.sync.dma_start(out=selp_s[:], in_=selp[:])
            selv_s = wpool.tile([128, G2 * group], BF16)
            nc.sync.dma_start(out=selv_s[:], in_=selv[:])
            bsels_s = wpool.tile([G2, R * group], BF16)
            nc.sync.dma_start(out=bsels_s[:], in_=bsels[:])
            bselv_s = wpool.tile([G2, 128 * group], BF16)
            nc.sync.dma_start(out=bselv_s[:], in_=bselv[:])

            for g in range(ngroups):
                g0 = g * group * NB  # column offset of this group
                # ================= phase A: v-dependent stats =================
                # v tiles arrive as two half-group transfers (16 KiB/partition
                # strips) and stay resident until phase B consumes them
                half = group // 2
                vt_h = []
                for h in range(2):
                    vh = vpool.tile([128, 2, half * NB], BF16, tag="vt")
                    h0 = g0 + h * half * NB
                    nc.sync.dma_start(
                        out=vh[:],
                        in_=vt[:, h0 : h0 + half * NB].rearrange(
                            "(c p) n -> p c n", p=128
                        ),
                    )
                    vt_h.append(vh)

                def vslice(j):
                    h = j // half
                    lo = (j % half) * NB
                    return vt_h[h], slice(lo, lo + NB)
                sqp_tiles = []
                v2_tiles = []
                st_ps = ps_stats.tile([G2, NB], F32, tag="stats")

                # select-matmuls accumulate block j's stats; lagged LAG blocks
                # behind the proj matmuls so the ACT squares are long done and
                # the PE never stalls mid-stream (keeps HAM at full clock).
                LAG = 2

                def emit_sel(j):
                    nc.tensor.matmul(
                        st_ps[:],
                        selp_s[:, G2 * j : G2 * (j + 1)],
                        sqp_tiles[j][:],
                        start=(j == 0),
                        stop=False,
                    )
                    for c in range(2):
                        nc.tensor.matmul(
                            st_ps[:],
                            selv_s[:, G2 * j : G2 * (j + 1)],
                            v2_tiles[j][:, c, :],
                            start=False,
                            stop=(j == group - 1 and c == 1),
                        )

                for j in range(group):
                    vtile, jsl = vslice(j)
                    # proj^T = U.T @ v^T   [R, NB]
                    proj_ps = ps_proj.tile([R, NB], F32)
                    for c in range(2):
                        nc.tensor.matmul(
                            proj_ps[:],
                            u_s[:, c, :],
                            vtile[:, c, jsl],
                            start=(c == 0),
                            stop=(c == 1),
                        )
                    # proj^2 (Square is available in every ACT table set)
                    sqp_t = sqp_pool.tile([R, NB], BF16)
                    nc.scalar.activation(sqp_t[:], proj_ps[:], AF.Square)
                    sqp_tiles.append(sqp_t)
                    # v^2 (consumed by the lagged select-matmuls)
                    v2_t = v2_pool.tile([128, 2, NB], BF16)
                    nc.scalar.activation(v2_t[:], vtile[:, :, jsl], AF.Square)
                    v2_tiles.append(v2_t)
                    if j >= LAG:
                        emit_sel(j - LAG)
                for j in range(group - LAG, group):
                    emit_sel(j)

                # ---- group stats chain (rows 0..G-1: ||proj||, G..2G-1: ||v||)
                sq_t = st_pool.tile([G2, NB], F32, tag="st")
                nc.scalar.activation(sq_t[:], st_ps[:], AF.Sqrt)
                den_t = st_pool.tile([G2, NB], F32, tag="st")
                nc.vector.tensor_scalar(den_t[:], sq_t[:], 1.0 + EPS, None, OP.add)
                rec_t = st_pool.tile([G2, NB], F32, tag="st")
                nc.vector.reciprocal_approx_fast(rec_t[:], den_t[:])
                scale_t = st_pool.tile([G2, NB], BF16, tag="stb")
                with nc.allow_low_precision("bf16 matmul operand"):
                    nc.vector.tensor_copy(scale_t[:], rec_t[:])
                velf_t = st_pool.tile([G2, NB], BF16, tag="stb")
                nc.vector.tensor_scalar(velf_t[:], sq_t[:], VEL_A, VEL_B, OP.mult, OP.add)

                # ================= phase B: gates + finale =================
                for j in range(group):
                    b0 = g0 + j * NB
                    vtile, jsl = vslice(j)
                    sqp_t = sqp_tiles[j]

                    if j % XF_BATCH == 0:
                        xt_t = xfpool.tile([128, 2, XF_BATCH * NB], BF16, tag="x")
                        nc.sync.dma_start(
                            out=xt_t[:],
                            in_=xt[:, b0 : b0 + XF_BATCH * NB].rearrange(
                                "(c p) n -> p c n", p=128
                            ),
                        )
                        ft_t = xfpool.tile([128, 2, XF_BATCH * NB], BF16, tag="f")
                        nc.sync.dma_start(
                            out=ft_t[:],
                            in_=ft[:, b0 : b0 + XF_BATCH * NB].rearrange(
                                "(c p) n -> p c n", p=128
                            ),
                        )
                    ksl = slice((j % XF_BATCH) * NB, (j % XF_BATCH + 1) * NB)

                    # gates^T[d] = Wf @ x^T + Wi @ f^T  (accumulate 4 matmuls)
                    sig_ts = []
                    for d in range(2):
                        g_ps = ps_mm.tile([128, NB], F32, tag="mm")
                        dsl = slice(128 * d, 128 * (d + 1))
                        for c in range(2):
                            nc.tensor.matmul(
                                g_ps[:],
                                wft_s[:, c, dsl],
                                xt_t[:, c, ksl],
                                start=(c == 0),
                                stop=False,
                            )
                        for c in range(2):
                            nc.tensor.matmul(
                                g_ps[:],
                                wit_s[:, c, dsl],
                                ft_t[:, c, ksl],
                                start=False,
                                stop=(c == 1),
                            )
                        # mu_base^T = sigmoid(gates^T + bf)
                        sig_t = sig_pool.tile([128, NB], BF16, tag="sig")
                        nc.scalar.activation(
                            sig_t[:], g_ps[:], AF.Sigmoid, bias=bf_s[:, d : d + 1]
                        )
                        sig_ts.append(sig_t)

                    # broadcast scale row j -> [R, NB]; pre-scale proj^2
                    bc16_ps = ps_bc16.tile([R, NB], F32)
                    nc.tensor.matmul(
                        bc16_ps[:], bsels_s[:, R * j : R * (j + 1)], scale_t[:]
                    )
                    sqs_t = sqs_pool.tile([R, NB], BF16, tag="sqs")
                    with nc.allow_low_precision("bf16 matmul operand"):
                        nc.vector.tensor_tensor(sqs_t[:], sqp_t[:], bc16_ps[:], OP.mult)
                    # broadcast velocity-friction row group+j -> [128, NB]
                    velf_ps = ps_bcv.tile([128, NB], F32)
                    nc.tensor.matmul(
                        velf_ps[:], bselv_s[:, 128 * j : 128 * (j + 1)], velf_t[:]
                    )

                    if j % OUT_BATCH == 0:
                        th_t = out_pool.tile([128, 2, OUT_BATCH * NB], BF16, tag="th")
                    osl = slice((j % OUT_BATCH) * NB, (j % OUT_BATCH + 1) * NB)
                    for d in range(2):
                        # gamma^T[d] = (W.T chunk) @ (proj^2 * scale)
                        gam_ps = ps_mm.tile([128, NB], F32, tag="mm")
                        nc.tensor.matmul(
                            gam_ps[:], wg_s[:, 128 * d : 128 * (d + 1)], sqs_t[:]
                        )
                        # mu = sig * velf ; X = gamma + mu * v
                        mu2_t = mu_pool.tile([128, NB], BF16, tag="mu2")
                        with nc.allow_low_precision("bf16 intermediate"):
                            nc.vector.tensor_tensor(
                                mu2_t[:], sig_ts[d][:], velf_ps[:], OP.mult
                            )
                        mu3_t = mu_pool.tile([128, NB], BF16, tag="mu3")
                        with nc.allow_low_precision("bf16 intermediate"):
                            nc.vector.tensor_tensor(
                                mu3_t[:], mu2_t[:], vtile[:, d, jsl], OP.mult
                            )
                        nc.vector.tensor_tensor(gam_ps[:], gam_ps[:], mu3_t[:], OP.add)
                        # out = 10 * tanh(X / 10): tanh(0.1 x) here, x10 below
                        nc.scalar.activation(
                            th_t[:, d, osl], gam_ps[:], AF.Tanh, scale=1.0 / CLAMP
                        )
                    if j % OUT_BATCH == OUT_BATCH - 1:
                        with nc.allow_low_precision("fp16 output tile"):
                            nc.vector.tensor_scalar(
                                th_t[:], th_t[:], CLAMP, None, OP.mult
                            )
                        ob = b0 - (OUT_BATCH - 1) * NB
                        # ACT HWDGE ring: separate FIFO from the input (SP)
                        # ring, immune to SWDGE descriptor starvation
                        nc.scalar.dma_start(
                            out=outt[:, ob : ob + OUT_BATCH * NB].rearrange(
                                "(c p) n -> p c n", p=128
                            ),
                            in_=th_t[:],
                        )

    nc.finalize()
    return nc


_BUILD_CACHE = {}


def _get_module(rows_per_core: int, group: int):
    key = (rows_per_core, group)
    if key not in _BUILD_CACHE:
        _BUILD_CACHE[key] = build(rows_per_core, group)
    return _BUILD_CACHE[key]


def _make_in_maps(v, x, force, U, W, Wf, bf, Wi, rows_per_core, group):
    bff = np.float16
    G2 = 2 * group
    wft = np.ascontiguousarray(Wf.T).astype(bff)
    wit = np.ascontiguousarray(Wi.T).astype(bff)
    u = np.ascontiguousarray(U).astype(bff)
    wg = np.ascontiguousarray(W.T).astype(bff)
    bf_c = np.ascontiguousarray(bf).astype(np.float32, copy=False)
    selp = np.zeros((R, G2 * group), bff)
    selv = np.zeros((128, G2 * group), bff)
    bsels = np.zeros((G2, R * group), bff)
    bselv = np.zeros((G2, 128 * group), bff)
    for j in range(group):
        selp[:, G2 * j + j] = 1.0
        selv[:, G2 * j + group + j] = 1.0
        bsels[j, R * j : R * (j + 1)] = 1.0
        bselv[group + j, 128 * j : 128 * (j + 1)] = 1.0

    ncores = v.shape[0] // rows_per_core
    in_maps = []
    for c in range(ncores):
        sl = slice(c * rows_per_core, (c + 1) * rows_per_core)
        in_maps.append(
            {
                "vt": v[sl].T.astype(bff),
                "xt": x[sl].T.astype(bff),
                "ft": force[sl].T.astype(bff),
                "wft": wft,
                "wit": wit,
                "u": u,
                "wg": wg,
                "bf": bf_c,
                "selp": selp,
                "selv": selv,
                "bsels": bsels,
                "bselv": bselv,
            }
        )
    return in_maps


def run(v, x, force, U, W, Wf, bf, Wi, rows_per_core, group, trace=False, **kw):
    nc = _get_module(rows_per_core, group)
    in_maps = _make_in_maps(v, x, force, U, W, Wf, bf, Wi, rows_per_core, group)
    ncores = len(in_maps)
    res = run_bass_kernel_spmd(
        nc, in_maps, core_ids=list(range(ncores)), trace=trace, **kw
    )
    n = v.shape[0]
    out = np.empty((n, D), np.float32)
    for c in range(ncores):
        out[c * rows_per_core : (c + 1) * rows_per_core, :] = res.results[c]["outt"].T.astype(np.float32)
    return out, res


def kernel(v, x, force, U, W, Wf, bf, Wi):
    out, _ = run(v, x, force, U, W, Wf, bf, Wi, ROWS_PER_CORE, 16)
    return out


# revision 13
# speedup vs baseline: 1.0728x; 1.0728x over previous
"""Trainium2 Bass kernel for nn_LowRankChristoffel (8 NeuronCores, data parallel).

Reference computation (per row b of v/x/force, D=256, R=16):
    proj  = v @ U                           [B, R]
    scale = 1 / (1 + ||proj|| + eps)
    gamma = (proj^2 * scale) @ W.T          [B, D]
    gate  = x @ Wf.T + bf + force @ Wi.T    [B, D]
    mu    = sigmoid(gate) * 0.1 * (1 + 0.1 * ||v|| / (sqrt(D) + eps))
    out   = 10 * tanh((gamma + mu * v) / 10)

Strategy:
  - Pure data parallel over 8 cores (batch 262144 -> 32768 rows/core).
  - Host pre-transposes v/x/force shards to [D, rows] (and casts to fp16)
    so every SBUF tile arrives in [dim, row] layout (contraction dim on
    partitions) with zero on-chip transposes; output is produced
    transposed (f32) and un-transposed on the host.
  - All matmuls in fp16 (1 cyc/row), accumulating in f32 PSUM.
  - Per-row statistics (||v||^2 rows G..2G-1, ||proj||^2 rows 0..G-1) are
    accumulated into one batched [2G, 512] PSUM tile via "select-column"
    matmuls; a single Sqrt + reciprocal chain runs once per G-block group
    (exactly 2 ACT table loads per group: sqrt set <-> sigmoid/tanh set).
  - Per-row scalars are broadcast to [*, 512] tiles with row-select
    matmuls straight from the batched stats tiles.
"""

import sys
import types

import numpy as np

sys.path.insert(0, "/opt/trn_rl_repo")

import concourse.mybir as mybir
import concourse.tile as tile
from concourse import bacc


def _install_ntff_shim():
    """Register the NTFF profile hook that trn_boot skips when
    antenv.axon_hooks is absent from the image (needed for trace=True)."""
    if "antenv.axon_hooks" in sys.modules:
        return
    try:
        from trn_agent_boot.trn_boot import _ntff_profile_via_ctypes

        hook = _ntff_profile_via_ctypes("/opt/axon/libaxon_pjrt.so")
    except Exception:
        hook = None
    mod = types.ModuleType("antenv.axon_hooks")
    mod._hook = hook
    mod.get_axon_ntff_profile_hook = lambda: mod._hook
    mod.set_axon_ntff_profile_hook = lambda h: setattr(mod, "_hook", h)
    sys.modules["antenv.axon_hooks"] = mod


_install_ntff_shim()
import ml_dtypes
from concourse.bass_utils import run_bass_kernel_spmd

NCORES = 8
D = 256
R = 16
BATCH = 262144
ROWS_PER_CORE = BATCH // NCORES  # 32768
NB = 512  # rows per block (matmul moving-dim)
XF_BATCH = 4  # blocks per x/force DMA
OUT_BATCH = 4  # blocks per output DMA

CLAMP = 10.0
FRICTION_SCALE = 0.1
EPS = 1e-6
VEL_SCALE = 0.1

F32 = mybir.dt.float32
BF16 = mybir.dt.float16  # 16-bit compute dtype (fp16: 11-bit mantissa)


def build(rows_per_core: int, group: int):
    """Build the Bass module for one core (SPMD across all cores)."""
    nblk = rows_per_core // NB
    assert nblk % group == 0
    ngroups = nblk // group
    assert group % XF_BATCH == 0 and group % OUT_BATCH == 0
    G2 = 2 * group
    assert G2 <= 128

    nc = bacc.Bacc(None, target_bir_lowering=False)

    vt = nc.declare_dram_parameter("vt", [D, rows_per_core], BF16, isOutput=False)
    xt = nc.declare_dram_parameter("xt", [D, rows_per_core], BF16, isOutput=False)
    ft = nc.declare_dram_parameter("ft", [D, rows_per_core], BF16, isOutput=False)
    wft = nc.declare_dram_parameter("wft", [D, D], BF16, isOutput=False)
    wit = nc.declare_dram_parameter("wit", [D, D], BF16, isOutput=False)
    u_p = nc.declare_dram_parameter("u", [D, R], BF16, isOutput=False)
    wg = nc.declare_dram_parameter("wg", [R, D], BF16, isOutput=False)
    bf = nc.declare_dram_parameter("bf", [D], F32, isOutput=False)
    selp = nc.declare_dram_parameter("selp", [R, G2 * group], BF16, isOutput=False)
    selv = nc.declare_dram_parameter("selv", [128, G2 * group], BF16, isOutput=False)
    bsels = nc.declare_dram_parameter("bsels", [G2, R * group], BF16, isOutput=False)
    bselv = nc.declare_dram_parameter("bselv", [G2, 128 * group], BF16, isOutput=False)
    outt = nc.declare_dram_parameter("outt", [D, rows_per_core], BF16, isOutput=True)

    AF = mybir.ActivationFunctionType
    OP = mybir.AluOpType
    VEL_A = float(FRICTION_SCALE * VEL_SCALE / (np.sqrt(D) + EPS))
    VEL_B = float(FRICTION_SCALE)

    with tile.TileContext(nc) as tc:
        with (
            tc.tile_pool(name="wpool", bufs=1) as wpool,
            tc.tile_pool(name="vpool", bufs=3) as vpool,
            tc.tile_pool(name="xfpool", bufs=2) as xfpool,
            tc.tile_pool(name="sqp", bufs=group + 2) as sqp_pool,
            tc.tile_pool(name="sqsp", bufs=3) as sqs_pool,
            tc.tile_pool(name="v2p", bufs=4) as v2_pool,
            tc.tile_pool(name="sigp", bufs=3) as sig_pool,
            tc.tile_pool(name="mup", bufs=3) as mu_pool,
            tc.tile_pool(name="outp", bufs=3) as out_pool,
            tc.tile_pool(name="stp", bufs=4) as st_pool,
            tc.tile_pool(name="ps_stats", bufs=1, space="PSUM") as ps_stats,
            tc.tile_pool(name="ps_proj", bufs=1, space="PSUM") as ps_proj,
            tc.tile_pool(name="ps_bc16", bufs=1, space="PSUM") as ps_bc16,
            tc.tile_pool(name="ps_mm", bufs=4, space="PSUM") as ps_mm,
            tc.tile_pool(name="ps_bcv", bufs=1, space="PSUM") as ps_bcv,
        ):
            # ---- load weights / constants once ----
            wft_s = wpool.tile([128, 2, D], BF16)
            nc.sync.dma_start(out=wft_s[:], in_=wft.rearrange("(c p) m -> p c m", p=128))
            wit_s = wpool.tile([128, 2, D], BF16)
            nc.sync.dma_start(out=wit_s[:], in_=wit.rearrange("(c p) m -> p c m", p=128))
            u_s = wpool.tile([128, 2, R], BF16)
            nc.sync.dma_start(out=u_s[:], in_=u_p.rearrange("(c p) r -> p c r", p=128))
            wg_s = wpool.tile([R, D], BF16)
            nc.sync.dma_start(out=wg_s[:], in_=wg[:])
            bf_s = wpool.tile([128, 2], F32)
            nc.sync.dma_start(out=bf_s[:], in_=bf.rearrange("(c p) -> p c", p=128))
            selp_s = wpool.tile([R, G2 * group], BF16)
            nc.sync.dma_start(out=selp_s[:], in_=selp[:])
            selv_s = wpool.tile([128, G2 * group], BF16)
            nc.sync.dma_start(out=selv_s[:], in_=selv[:])
            bsels_s = wpool.tile([G2, R * group], BF16)
            nc.sync.dma_start(out=bsels_s[:], in_=bsels[:])
            bselv_s = wpool.tile([G2, 128 * group], BF16)
            nc.sync.dma_start(out=bselv_s[:], in_=bselv[:])

            for g in range(ngroups):
                g0 = g * group * NB  # column offset of this group
                # ================= phase A: v-dependent stats =================
                # v tiles arrive as two half-group transfers (16 KiB/partition
                # strips) and stay resident until phase B consumes them
                half = group // 2
                vt_h = []
                for h in range(2):
                    vh = vpool.tile([128, 2, half * NB], BF16, tag="vt")
                    h0 = g0 + h * half * NB
                    nc.sync.dma_start(
                        out=vh[:],
                        in_=vt[:, h0 : h0 + half * NB].rearrange(
                            "(c p) n -> p c n", p=128
                        ),
                    )
                    vt_h.append(vh)

                def vslice(j):
                    h = j // half
                    lo = (j % half) * NB
                    return vt_h[h], slice(lo, lo + NB)
                sqp_tiles = []
                v2_tiles = []
                st_ps = ps_stats.tile([G2, NB], F32, tag="stats")

                # select-matmuls accumulate block j's stats; lagged LAG blocks
                # behind the proj matmuls so the ACT squares are long done and
                # the PE never stalls mid-stream (keeps HAM at full clock).
                LAG = 2

                def emit_sel(j):
                    nc.tensor.matmul(
                        st_ps[:],
                        selp_s[:, G2 * j : G2 * (j + 1)],
                        sqp_tiles[j][:],
                        start=(j == 0),
                        stop=False,
                    )
                    for c in range(2):
                        nc.tensor.matmul(
                            st_ps[:],
                            selv_s[:, G2 * j : G2 * (j + 1)],
                            v2_tiles[j][:, c, :],
                            start=False,
                            stop=(j == group - 1 and c == 1),
                        )

                for j in range(group):
                    vtile, jsl = vslice(j)
                    # proj^T = U.T @ v^T   [R, NB]
                    proj_ps = ps_proj.tile([R, NB], F32, tag="proj")
                    for c in range(2):
                        nc.tensor.matmul(
                            proj_ps[:],
                            u_s[:, c, :],
                            vtile[:, c, jsl],
                            start=(c == 0),
                            stop=(c == 1),
                        )
                    # proj^2 (Square is available in every ACT table set)
                    sqp_t = sqp_pool.tile([R, NB], BF16)
                    nc.scalar.activation(sqp_t[:], proj_ps[:], AF.Square)
                    sqp_tiles.append(sqp_t)
                    # v^2 (consumed by the lagged select-matmuls)
                    v2_t = v2_pool.tile([128, 2, NB], BF16)
                    nc.scalar.activation(v2_t[:], vtile[:, :, jsl], AF.Square)
                    v2_tiles.append(v2_t)
                    if j >= LAG:
                        emit_sel(j - LAG)
                for j in range(group - LAG, group):
                    emit_sel(j)

                # ---- group stats chain (rows 0..G-1: ||proj||, G..2G-1: ||v||)
                sq_t = st_pool.tile([G2, NB], F32, tag="st")
                nc.scalar.activation(sq_t[:], st_ps[:], AF.Sqrt)
                den_t = st_pool.tile([G2, NB], F32, tag="st")
                nc.vector.tensor_scalar(den_t[:], sq_t[:], 1.0 + EPS, None, OP.add)
                rec_t = st_pool.tile([G2, NB], F32, tag="st")
                nc.vector.reciprocal_approx_fast(rec_t[:], den_t[:])
                scale_t = st_pool.tile([G2, NB], BF16, tag="stb")
                with nc.allow_low_precision("bf16 matmul operand"):
                    nc.vector.tensor_copy(scale_t[:], rec_t[:])
                velf_t = st_pool.tile([G2, NB], BF16, tag="stb")
                nc.vector.tensor_scalar(velf_t[:], sq_t[:], VEL_A, VEL_B, OP.mult, OP.add)

                # ================= phase B: gates + finale =================
                def emit_scale_chain(j):
                    # broadcast scale row j -> [R, NB]; pre-scale proj^2.
                    # Emitted one block early so gamma(j) never waits on DVE.
                    bc16_ps = ps_bc16.tile([R, NB], F32, tag="bc16")
                    nc.tensor.matmul(
                        bc16_ps[:], bsels_s[:, R * j : R * (j + 1)], scale_t[:]
                    )
                    sqs_t = sqs_pool.tile([R, NB], BF16, tag="sqs")
                    with nc.allow_low_precision("bf16 matmul operand"):
                        nc.vector.tensor_tensor(
                            sqs_t[:], sqp_tiles[j][:], bc16_ps[:], OP.mult
                        )
                    return sqs_t

                sqs_next = emit_scale_chain(0)
                for j in range(group):
                    b0 = g0 + j * NB
                    vtile, jsl = vslice(j)
                    sqs_t = sqs_next

                    if j % XF_BATCH == 0:
                        xt_t = xfpool.tile([128, 2, XF_BATCH * NB], BF16, tag="x")
                        nc.sync.dma_start(
                            out=xt_t[:],
                            in_=xt[:, b0 : b0 + XF_BATCH * NB].rearrange(
                                "(c p) n -> p c n", p=128
                            ),
                        )
                        ft_t = xfpool.tile([128, 2, XF_BATCH * NB], BF16, tag="f")
                        nc.sync.dma_start(
                            out=ft_t[:],
                            in_=ft[:, b0 : b0 + XF_BATCH * NB].rearrange(
                                "(c p) n -> p c n", p=128
                            ),
                        )
                    ksl = slice((j % XF_BATCH) * NB, (j % XF_BATCH + 1) * NB)

                    # gates^T[d] = Wf @ x^T + Wi @ f^T  (accumulate 4 matmuls)
                    sig_ts = []
                    for d in range(2):
                        g_ps = ps_mm.tile([128, NB], F32, tag="mm")
                        dsl = slice(128 * d, 128 * (d + 1))
                        for c in range(2):
                            nc.tensor.matmul(
                                g_ps[:],
                                wft_s[:, c, dsl],
                                xt_t[:, c, ksl],
                                start=(c == 0),
                                stop=False,
                            )
                        for c in range(2):
                            nc.tensor.matmul(
                                g_ps[:],
                                wit_s[:, c, dsl],
                                ft_t[:, c, ksl],
                                start=False,
                                stop=(c == 1),
                            )
                        # mu_base^T = sigmoid(gates^T + bf)
                        sig_t = sig_pool.tile([128, NB], BF16, tag="sig")
                        nc.scalar.activation(
                            sig_t[:], g_ps[:], AF.Sigmoid, bias=bf_s[:, d : d + 1]
                        )
                        sig_ts.append(sig_t)

                    if j + 1 < group:
                        sqs_next = emit_scale_chain(j + 1)
                    # broadcast velocity-friction row group+j -> [128, NB]
                    velf_ps = ps_bcv.tile([128, NB], F32, tag="bcv")
                    nc.tensor.matmul(
                        velf_ps[:], bselv_s[:, 128 * j : 128 * (j + 1)], velf_t[:]
                    )

                    if j % OUT_BATCH == 0:
                        th_t = out_pool.tile([128, 2, OUT_BATCH * NB], BF16, tag="th")
                    osl = slice((j % OUT_BATCH) * NB, (j % OUT_BATCH + 1) * NB)
                    for d in range(2):
                        # gamma^T[d] = (W.T chunk) @ (proj^2 * scale)
                        gam_ps = ps_mm.tile([128, NB], F32, tag="mm")
                        nc.tensor.matmul(
                            gam_ps[:], wg_s[:, 128 * d : 128 * (d + 1)], sqs_t[:]
                        )
                        # mu = sig * velf ; X = gamma + mu * v
                        mu2_t = mu_pool.tile([128, NB], BF16, tag="mu2")
                        with nc.allow_low_precision("bf16 intermediate"):
                            nc.vector.tensor_tensor(
                                mu2_t[:], sig_ts[d][:], velf_ps[:], OP.mult
                            )
                        mu3_t = mu_pool.tile([128, NB], BF16, tag="mu3")
                        with nc.allow_low_precision("bf16 intermediate"):
                            nc.vector.tensor_tensor(
                                mu3_t[:], mu2_t[:], vtile[:, d, jsl], OP.mult
                            )
                        nc.vector.tensor_tensor(gam_ps[:], gam_ps[:], mu3_t[:], OP.add)
                        # out = 10 * tanh(X / 10): tanh(0.1 x) here, x10 below
                        nc.scalar.activation(
                            th_t[:, d, osl], gam_ps[:], AF.Tanh, scale=1.0 / CLAMP
                        )
                    if j % OUT_BATCH == OUT_BATCH - 1:
                        with nc.allow_low_precision("fp16 output tile"):
                            nc.vector.tensor_scalar(
                                th_t[:], th_t[:], CLAMP, None, OP.mult
                            )
                        ob = b0 - (OUT_BATCH - 1) * NB
                        # ACT HWDGE ring: separate FIFO from the input (SP)
                        # ring, immune to SWDGE descriptor starvation
                        nc.scalar.dma_start(
                            out=outt[:, ob : ob + OUT_BATCH * NB].rearrange(
                                "(c p) n -> p c n", p=128
                            ),
                            in_=th_t[:],
                        )

    nc.finalize()
    return nc


_BUILD_CACHE = {}


def _get_module(rows_per_core: int, group: int):
    key = (rows_per_core, group)
    if key not in _BUILD_CACHE:
        _BUILD_CACHE[key] = build(rows_per_core, group)
    return _BUILD_CACHE[key]


def _make_in_maps(v, x, force, U, W, Wf, bf, Wi, rows_per_core, group):
    bff = np.float16
    G2 = 2 * group
    wft = np.ascontiguousarray(Wf.T).astype(bff)
    wit = np.ascontiguousarray(Wi.T).astype(bff)
    u = np.ascontiguousarray(U).astype(bff)
    wg = np.ascontiguousarray(W.T).astype(bff)
    bf_c = np.ascontiguousarray(bf).astype(np.float32, copy=False)
    selp = np.zeros((R, G2 * group), bff)
    selv = np.zeros((128, G2 * group), bff)
    bsels = np.zeros((G2, R * group), bff)
    bselv = np.zeros((G2, 128 * group), bff)
    for j in range(group):
        selp[:, G2 * j + j] = 1.0
        selv[:, G2 * j + group + j] = 1.0
        bsels[j, R * j : R * (j + 1)] = 1.0
        bselv[group + j, 128 * j : 128 * (j + 1)] = 1.0

    ncores = v.shape[0] // rows_per_core
    in_maps = []
    for c in range(ncores):
        sl = slice(c * rows_per_core, (c + 1) * rows_per_core)
        in_maps.append(
            {
                "vt": v[sl].T.astype(bff),
                "xt": x[sl].T.astype(bff),
                "ft": force[sl].T.astype(bff),
                "wft": wft,
                "wit": wit,
                "u": u,
                "wg": wg,
                "bf": bf_c,
                "selp": selp,
                "selv": selv,
                "bsels": bsels,
                "bselv": bselv,
            }
        )
    return in_maps


def run(v, x, force, U, W, Wf, bf, Wi, rows_per_core, group, trace=False, **kw):
    nc = _get_module(rows_per_core, group)
    in_maps = _make_in_maps(v, x, force, U, W, Wf, bf, Wi, rows_per_core, group)
    ncores = len(in_maps)
    res = run_bass_kernel_spmd(
        nc, in_maps, core_ids=list(range(ncores)), trace=trace, **kw
    )
    n = v.shape[0]
    out = np.empty((n, D), np.float32)
    for c in range(ncores):
        out[c * rows_per_core : (c + 1) * rows_per_core, :] = res.results[c]["outt"].T.astype(np.float32)
    return out, res


def kernel(v, x, force, U, W, Wf, bf, Wi):
    out, _ = run(v, x, force, U, W, Wf, bf, Wi, ROWS_PER_CORE, 16)
    return out


# revision 14
# speedup vs baseline: 1.0958x; 1.0215x over previous
"""Trainium2 Bass kernel for nn_LowRankChristoffel (8 NeuronCores, data parallel).

Reference computation (per row b of v/x/force, D=256, R=16):
    proj  = v @ U                           [B, R]
    scale = 1 / (1 + ||proj|| + eps)
    gamma = (proj^2 * scale) @ W.T          [B, D]
    gate  = x @ Wf.T + bf + force @ Wi.T    [B, D]
    mu    = sigmoid(gate) * 0.1 * (1 + 0.1 * ||v|| / (sqrt(D) + eps))
    out   = 10 * tanh((gamma + mu * v) / 10)

Strategy:
  - Pure data parallel over 8 cores (batch 262144 -> 32768 rows/core).
  - Host pre-transposes v/x/force shards to [D, rows] (and casts to fp16)
    so every SBUF tile arrives in [dim, row] layout (contraction dim on
    partitions) with zero on-chip transposes; output is produced
    transposed (f32) and un-transposed on the host.
  - All matmuls in fp16 (1 cyc/row), accumulating in f32 PSUM.
  - Per-row statistics (||v||^2 rows G..2G-1, ||proj||^2 rows 0..G-1) are
    accumulated into one batched [2G, 512] PSUM tile via "select-column"
    matmuls; a single Sqrt + reciprocal chain runs once per G-block group
    (exactly 2 ACT table loads per group: sqrt set <-> sigmoid/tanh set).
  - Per-row scalars are broadcast to [*, 512] tiles with row-select
    matmuls straight from the batched stats tiles.
"""

import sys
import types

import numpy as np

sys.path.insert(0, "/opt/trn_rl_repo")

import concourse.mybir as mybir
import concourse.tile as tile
from concourse import bacc


def _install_ntff_shim():
    """Register the NTFF profile hook that trn_boot skips when
    antenv.axon_hooks is absent from the image (needed for trace=True)."""
    if "antenv.axon_hooks" in sys.modules:
        return
    try:
        from trn_agent_boot.trn_boot import _ntff_profile_via_ctypes

        hook = _ntff_profile_via_ctypes("/opt/axon/libaxon_pjrt.so")
    except Exception:
        hook = None
    mod = types.ModuleType("antenv.axon_hooks")
    mod._hook = hook
    mod.get_axon_ntff_profile_hook = lambda: mod._hook
    mod.set_axon_ntff_profile_hook = lambda h: setattr(mod, "_hook", h)
    sys.modules["antenv.axon_hooks"] = mod


_install_ntff_shim()
import ml_dtypes
from concourse.bass_utils import run_bass_kernel_spmd

NCORES = 8
D = 256
R = 16
BATCH = 262144
ROWS_PER_CORE = BATCH // NCORES  # 32768
NB = 512  # rows per block (matmul moving-dim)
XF_BATCH = 4  # blocks per x/force DMA
OUT_BATCH = 4  # blocks per output DMA

CLAMP = 10.0
FRICTION_SCALE = 0.1
EPS = 1e-6
VEL_SCALE = 0.1

F32 = mybir.dt.float32
BF16 = mybir.dt.float16  # 16-bit compute dtype (fp16: 11-bit mantissa)


def build(rows_per_core: int, group: int):
    """Build the Bass module for one core (SPMD across all cores)."""
    nblk = rows_per_core // NB
    assert nblk % group == 0
    ngroups = nblk // group
    assert group % XF_BATCH == 0 and group % OUT_BATCH == 0
    G2 = 2 * group
    assert G2 <= 128

    nc = bacc.Bacc(None, target_bir_lowering=False)

    vt = nc.declare_dram_parameter("vt", [D, rows_per_core], BF16, isOutput=False)
    xt = nc.declare_dram_parameter("xt", [D, rows_per_core], BF16, isOutput=False)
    ft = nc.declare_dram_parameter("ft", [D, rows_per_core], BF16, isOutput=False)
    wft = nc.declare_dram_parameter("wft", [D, D], BF16, isOutput=False)
    wit = nc.declare_dram_parameter("wit", [D, D], BF16, isOutput=False)
    u_p = nc.declare_dram_parameter("u", [D, R], BF16, isOutput=False)
    wg = nc.declare_dram_parameter("wg", [R, D], BF16, isOutput=False)
    bf = nc.declare_dram_parameter("bf", [D], F32, isOutput=False)
    selp = nc.declare_dram_parameter("selp", [R, G2 * group], BF16, isOutput=False)
    selv = nc.declare_dram_parameter("selv", [128, G2 * group], BF16, isOutput=False)
    bsels = nc.declare_dram_parameter("bsels", [G2, R * group], BF16, isOutput=False)
    bselv = nc.declare_dram_parameter("bselv", [G2, 128 * group], BF16, isOutput=False)
    outt = nc.declare_dram_parameter("outt", [D, rows_per_core], BF16, isOutput=True)

    AF = mybir.ActivationFunctionType
    OP = mybir.AluOpType
    VEL_A = float(FRICTION_SCALE * VEL_SCALE / (np.sqrt(D) + EPS))
    VEL_B = float(FRICTION_SCALE)

    with tile.TileContext(nc) as tc:
        with (
            tc.tile_pool(name="wpool", bufs=1) as wpool,
            tc.tile_pool(name="vpool", bufs=3) as vpool,
            tc.tile_pool(name="xfpool", bufs=2) as xfpool,
            tc.tile_pool(name="sqp", bufs=group + 2) as sqp_pool,
            tc.tile_pool(name="sqsp", bufs=3) as sqs_pool,
            tc.tile_pool(name="v2p", bufs=4) as v2_pool,
            tc.tile_pool(name="sigp", bufs=3) as sig_pool,
            tc.tile_pool(name="mup", bufs=3) as mu_pool,
            tc.tile_pool(name="outp", bufs=3) as out_pool,
            tc.tile_pool(name="stp", bufs=4) as st_pool,
            tc.tile_pool(name="ps_stats", bufs=1, space="PSUM") as ps_stats,
            tc.tile_pool(name="ps_proj", bufs=1, space="PSUM") as ps_proj,
            tc.tile_pool(name="ps_bc16", bufs=1, space="PSUM") as ps_bc16,
            tc.tile_pool(name="ps_mm", bufs=4, space="PSUM") as ps_mm,
            tc.tile_pool(name="ps_bcv", bufs=1, space="PSUM") as ps_bcv,
        ):
            # ---- load weights / constants once ----
            wft_s = wpool.tile([128, 2, D], BF16)
            nc.sync.dma_start(out=wft_s[:], in_=wft.rearrange("(c p) m -> p c m", p=128))
            wit_s = wpool.tile([128, 2, D], BF16)
            nc.sync.dma_start(out=wit_s[:], in_=wit.rearrange("(c p) m -> p c m", p=128))
            u_s = wpool.tile([128, 2, R], BF16)
            nc.sync.dma_start(out=u_s[:], in_=u_p.rearrange("(c p) r -> p c r", p=128))
            wg_s = wpool.tile([R, D], BF16)
            nc.sync.dma_start(out=wg_s[:], in_=wg[:])
            bf_s = wpool.tile([128, 2], F32)
            nc.sync.dma_start(out=bf_s[:], in_=bf.rearrange("(c p) -> p c", p=128))
            selp_s = wpool.tile([R, G2 * group], BF16)
            nc.sync.dma_start(out=selp_s[:], in_=selp[:])
            selv_s = wpool.tile([128, G2 * group], BF16)
            nc.sync.dma_start(out=selv_s[:], in_=selv[:])
            bsels_s = wpool.tile([G2, R * group], BF16)
            nc.sync.dma_start(out=bsels_s[:], in_=bsels[:])
            bselv_s = wpool.tile([G2, 128 * group], BF16)
            nc.sync.dma_start(out=bselv_s[:], in_=bselv[:])

            for g in range(ngroups):
                g0 = g * group * NB  # column offset of this group
                # ================= phase A: v-dependent stats =================
                # v tiles arrive as two half-group transfers (16 KiB/partition
                # strips) and stay resident until phase B consumes them
                half = group // 2
                vt_h = []
                for h in range(2):
                    vh = vpool.tile([128, 2, half * NB], BF16, tag="vt")
                    h0 = g0 + h * half * NB
                    nc.sync.dma_start(
                        out=vh[:],
                        in_=vt[:, h0 : h0 + half * NB].rearrange(
                            "(c p) n -> p c n", p=128
                        ),
                    )
                    vt_h.append(vh)

                def vslice(j):
                    h = j // half
                    lo = (j % half) * NB
                    return vt_h[h], slice(lo, lo + NB)
                sqp_tiles = []
                v2_tiles = []
                st_ps = ps_stats.tile([G2, NB], F32, tag="stats")

                # select-matmuls accumulate block j's stats; lagged LAG blocks
                # behind the proj matmuls so the ACT squares are long done and
                # the PE never stalls mid-stream (keeps HAM at full clock).
                LAG = 2

                def emit_sel(j):
                    nc.tensor.matmul(
                        st_ps[:],
                        selp_s[:, G2 * j : G2 * (j + 1)],
                        sqp_tiles[j][:],
                        start=(j == 0),
                        stop=False,
                    )
                    for c in range(2):
                        nc.tensor.matmul(
                            st_ps[:],
                            selv_s[:, G2 * j : G2 * (j + 1)],
                            v2_tiles[j][:, c, :],
                            start=False,
                            stop=(j == group - 1 and c == 1),
                        )

                for j in range(group):
                    vtile, jsl = vslice(j)
                    # proj^T = U.T @ v^T   [R, NB]
                    proj_ps = ps_proj.tile([R, NB], F32, tag="proj")
                    for c in range(2):
                        nc.tensor.matmul(
                            proj_ps[:],
                            u_s[:, c, :],
                            vtile[:, c, jsl],
                            start=(c == 0),
                            stop=(c == 1),
                        )
                    # proj^2 (Square is available in every ACT table set)
                    sqp_t = sqp_pool.tile([R, NB], BF16)
                    nc.scalar.activation(sqp_t[:], proj_ps[:], AF.Square)
                    sqp_tiles.append(sqp_t)
                    # v^2 (consumed by the lagged select-matmuls)
                    v2_t = v2_pool.tile([128, 2, NB], BF16)
                    nc.scalar.activation(v2_t[:], vtile[:, :, jsl], AF.Square)
                    v2_tiles.append(v2_t)
                    if j >= LAG:
                        emit_sel(j - LAG)
                for j in range(group - LAG, group):
                    emit_sel(j)

                # ---- group stats chain (rows 0..G-1: ||proj||, G..2G-1: ||v||)
                sq_t = st_pool.tile([G2, NB], F32, tag="st")
                nc.scalar.activation(sq_t[:], st_ps[:], AF.Sqrt)
                den_t = st_pool.tile([G2, NB], F32, tag="st")
                nc.vector.tensor_scalar(den_t[:], sq_t[:], 1.0 + EPS, None, OP.add)
                rec_t = st_pool.tile([G2, NB], F32, tag="st")
                nc.vector.reciprocal_approx_fast(rec_t[:], den_t[:])
                scale_t = st_pool.tile([G2, NB], BF16, tag="stb")
                with nc.allow_low_precision("bf16 matmul operand"):
                    nc.vector.tensor_copy(scale_t[:], rec_t[:])
                velf_t = st_pool.tile([G2, NB], BF16, tag="stb")
                nc.vector.tensor_scalar(velf_t[:], sq_t[:], VEL_A, VEL_B, OP.mult, OP.add)

                # ================= phase B: gates + finale =================
                def emit_scale_chain(j):
                    # broadcast scale row j -> [R, NB]; pre-scale proj^2.
                    # Emitted one block early so gamma(j) never waits on DVE.
                    bc16_ps = ps_bc16.tile([R, NB], F32, tag="bc16")
                    nc.tensor.matmul(
                        bc16_ps[:], bsels_s[:, R * j : R * (j + 1)], scale_t[:]
                    )
                    sqs_t = sqs_pool.tile([R, NB], BF16, tag="sqs")
                    with nc.allow_low_precision("bf16 matmul operand"):
                        nc.vector.tensor_tensor(
                            sqs_t[:], sqp_tiles[j][:], bc16_ps[:], OP.mult
                        )
                    return sqs_t

                sqs_next = emit_scale_chain(0)
                for j in range(group):
                    b0 = g0 + j * NB
                    vtile, jsl = vslice(j)
                    sqs_t = sqs_next

                    if j % XF_BATCH == 0:
                        xt_t = xfpool.tile([128, 2, XF_BATCH * NB], BF16, tag="x")
                        nc.sync.dma_start(
                            out=xt_t[:],
                            in_=xt[:, b0 : b0 + XF_BATCH * NB].rearrange(
                                "(c p) n -> p c n", p=128
                            ),
                        )
                        ft_t = xfpool.tile([128, 2, XF_BATCH * NB], BF16, tag="f")
                        nc.sync.dma_start(
                            out=ft_t[:],
                            in_=ft[:, b0 : b0 + XF_BATCH * NB].rearrange(
                                "(c p) n -> p c n", p=128
                            ),
                        )
                    ksl = slice((j % XF_BATCH) * NB, (j % XF_BATCH + 1) * NB)

                    # gates^T[d] = Wf @ x^T + Wi @ f^T  (accumulate 4 matmuls)
                    sig_t = sig_pool.tile([128, 2, NB], BF16, tag="sig")
                    for d in range(2):
                        g_ps = ps_mm.tile([128, NB], F32, tag="mm")
                        dsl = slice(128 * d, 128 * (d + 1))
                        for c in range(2):
                            nc.tensor.matmul(
                                g_ps[:],
                                wft_s[:, c, dsl],
                                xt_t[:, c, ksl],
                                start=(c == 0),
                                stop=False,
                            )
                        for c in range(2):
                            nc.tensor.matmul(
                                g_ps[:],
                                wit_s[:, c, dsl],
                                ft_t[:, c, ksl],
                                start=False,
                                stop=(c == 1),
                            )
                        # mu_base^T = sigmoid(gates^T + bf)
                        nc.scalar.activation(
                            sig_t[:, d, :], g_ps[:], AF.Sigmoid, bias=bf_s[:, d : d + 1]
                        )

                    if j + 1 < group:
                        sqs_next = emit_scale_chain(j + 1)
                    # broadcast velocity-friction row group+j -> [128, NB]
                    velf_ps = ps_bcv.tile([128, NB], F32, tag="bcv")
                    nc.tensor.matmul(
                        velf_ps[:], bselv_s[:, 128 * j : 128 * (j + 1)], velf_t[:]
                    )

                    if j % OUT_BATCH == 0:
                        th_t = out_pool.tile([128, 2, OUT_BATCH * NB], BF16, tag="th")
                    osl = slice((j % OUT_BATCH) * NB, (j % OUT_BATCH + 1) * NB)
                    # mu = sig * velf ; mu3 = mu * v  (both d-tiles per op)
                    mu2_t = mu_pool.tile([128, 2, NB], BF16, tag="mu2")
                    with nc.allow_low_precision("bf16 intermediate"):
                        nc.vector.tensor_tensor(
                            mu2_t[:],
                            sig_t[:],
                            velf_ps[:, None, :].to_broadcast((128, 2, NB)),
                            OP.mult,
                        )
                    mu3_t = mu_pool.tile([128, 2, NB], BF16, tag="mu3")
                    with nc.allow_low_precision("bf16 intermediate"):
                        nc.vector.tensor_tensor(
                            mu3_t[:], mu2_t[:], vtile[:, :, jsl], OP.mult
                        )
                    for d in range(2):
                        # gamma^T[d] = (W.T chunk) @ (proj^2 * scale)
                        gam_ps = ps_mm.tile([128, NB], F32, tag="mm")
                        nc.tensor.matmul(
                            gam_ps[:], wg_s[:, 128 * d : 128 * (d + 1)], sqs_t[:]
                        )
                        # X = gamma + mu3, accumulated in place in PSUM
                        nc.vector.tensor_tensor(
                            gam_ps[:], gam_ps[:], mu3_t[:, d, :], OP.add
                        )
                        # out = 10 * tanh(X / 10): tanh(0.1 x) here, x10 below
                        nc.scalar.activation(
                            th_t[:, d, osl], gam_ps[:], AF.Tanh, scale=1.0 / CLAMP
                        )
                    if j % OUT_BATCH == OUT_BATCH - 1:
                        with nc.allow_low_precision("fp16 output tile"):
                            nc.vector.tensor_scalar(
                                th_t[:], th_t[:], CLAMP, None, OP.mult
                            )
                        ob = b0 - (OUT_BATCH - 1) * NB
                        # ACT HWDGE ring: separate FIFO from the input (SP)
                        # ring, immune to SWDGE descriptor starvation
                        nc.scalar.dma_start(
                            out=outt[:, ob : ob + OUT_BATCH * NB].rearrange(
                                "(c p) n -> p c n", p=128
                            ),
                            in_=th_t[:],
                        )

    nc.finalize()
    return nc


_BUILD_CACHE = {}


def _get_module(rows_per_core: int, group: int):
    key = (rows_per_core, group)
    if key not in _BUILD_CACHE:
        _BUILD_CACHE[key] = build(rows_per_core, group)
    return _BUILD_CACHE[key]


def _make_in_maps(v, x, force, U, W, Wf, bf, Wi, rows_per_core, group):
    bff = np.float16
    G2 = 2 * group
    wft = np.ascontiguousarray(Wf.T).astype(bff)
    wit = np.ascontiguousarray(Wi.T).astype(bff)
    u = np.ascontiguousarray(U).astype(bff)
    wg = np.ascontiguousarray(W.T).astype(bff)
    bf_c = np.ascontiguousarray(bf).astype(np.float32, copy=False)
    selp = np.zeros((R, G2 * group), bff)
    selv = np.zeros((128, G2 * group), bff)
    bsels = np.zeros((G2, R * group), bff)
    bselv = np.zeros((G2, 128 * group), bff)
    for j in range(group):
        selp[:, G2 * j + j] = 1.0
        selv[:, G2 * j + group + j] = 1.0
        bsels[j, R * j : R * (j + 1)] = 1.0
        bselv[group + j, 128 * j : 128 * (j + 1)] = 1.0

    ncores = v.shape[0] // rows_per_core
    in_maps = []
    for c in range(ncores):
        sl = slice(c * rows_per_core, (c + 1) * rows_per_core)
        in_maps.append(
            {
                "vt": v[sl].T.astype(bff),
                "xt": x[sl].T.astype(bff),
                "ft": force[sl].T.astype(bff),
                "wft": wft,
                "wit": wit,
                "u": u,
                "wg": wg,
                "bf": bf_c,
                "selp": selp,
                "selv": selv,
                "bsels": bsels,
                "bselv": bselv,
            }
        )
    return in_maps


def run(v, x, force, U, W, Wf, bf, Wi, rows_per_core, group, trace=False, **kw):
    nc = _get_module(rows_per_core, group)
    in_maps = _make_in_maps(v, x, force, U, W, Wf, bf, Wi, rows_per_core, group)
    ncores = len(in_maps)
    res = run_bass_kernel_spmd(
        nc, in_maps, core_ids=list(range(ncores)), trace=trace, **kw
    )
    n = v.shape[0]
    out = np.empty((n, D), np.float32)
    for c in range(ncores):
        out[c * rows_per_core : (c + 1) * rows_per_core, :] = res.results[c]["outt"].T.astype(np.float32)
    return out, res


def kernel(v, x, force, U, W, Wf, bf, Wi):
    out, _ = run(v, x, force, U, W, Wf, bf, Wi, ROWS_PER_CORE, 16)
    return out
